# revision 44
# baseline (speedup 1.0000x reference)
"""Trainium2 Bass kernel for nn_ConditionalSpline1DFlow (K=16 RQS flow).

Data-parallel over 8 cores (B=4096 -> 512 rows/core). Per core:
  1. Conditioner MLP on TensorE (feature-major).
  2. Spline params per row; rescale bin k's rational-quadratic by
     s_k = delta_0/delta_k so numerator N, denominator D and
     derivative-numerator C become globally CONTINUOUS piecewise
     quadratics in x.
  3. Evaluate N, D, C gather-free in the clipped-ramp basis
        P(x) = const + sum_k a_k*(t_k - x_k)^2 + b_k*(t_k - x_k),
        t_k = clip(x, x_k, x_{k+1})
     on TensorE: rows packed (b*16+k) so one [128, 24] matmul contracts
     all 16 bins x 3 polys for 8 batch rows at once; PSUM accumulates the
     (linear, square) stream pair.
  4. out = N/D + (y - clip(y)); logdet = (ln C - 2 ln D) * (y == clip(y)).
"""
import sys
import numpy as np

K = 16
BOUND = 5.0
MBW = 1e-3
MBH = 1e-3
MD = 1e-3
B_FULL, N = 4096, 1024
CD, H = 64, 256
OUT3 = 3 * K + 1
NCORES = 8
BL = B_FULL // NCORES   # 512 rows per core
T = BL // 128           # 4 partition tiles
G = 128 // 8            # (unused) 8-row groups
GG = 128 // 16          # 8 groups of 16 rows per tile
CH = N // 512           # 2 free-dim chunks

MODE = "u"  # "t": stream clipped-t w/ folded consts; "u": stream t - x_k
            # (u-basis: streams/coefs stay O(per-bin contribution), required
            # for fp32r's ~2^-12 rounding to stay inside the error budget)

_CACHE = {}


def _ensure_path():
    for p in ("/opt/trn_rl_repo",):
        if p not in sys.path:
            sys.path.insert(0, p)


def _build_nc():
    _ensure_path()
    import concourse.bacc as bacc
    import concourse.tile as tile
    from concourse import mybir

    fp32 = mybir.dt.float32
    nc = bacc.Bacc("TRN2", target_bir_lowering=False, debug=False)

    io = dict(
        cond=nc.dram_tensor("cond", [BL, CD], fp32, kind="ExternalInput"),
        y=nc.dram_tensor("y", [BL, N], fp32, kind="ExternalInput"),
        W1=nc.dram_tensor("W1", [CD, H], fp32, kind="ExternalInput"),
        b1=nc.dram_tensor("b1", [H], fp32, kind="ExternalInput"),
        W2=nc.dram_tensor("W2", [H, H], fp32, kind="ExternalInput"),
        b2=nc.dram_tensor("b2", [H], fp32, kind="ExternalInput"),
        W3=nc.dram_tensor("W3", [H, OUT3], fp32, kind="ExternalInput"),
        b3=nc.dram_tensor("b3", [OUT3], fp32, kind="ExternalInput"),
        out=nc.dram_tensor("out", [BL, N], fp32, kind="ExternalOutput"),
        logdet=nc.dram_tensor("logdet", [BL, N], fp32, kind="ExternalOutput"),
    )
    with tile.TileContext(nc) as tc:
        _emit(nc, tc, io)
    nc.compile()
    return nc


def _emit(nc, tc, io):
    from contextlib import ExitStack
    import concourse.bass as bass
    from concourse import mybir

    fp32 = mybir.dt.float32
    i32 = mybir.dt.int32
    AF = mybir.ActivationFunctionType
    OP = mybir.AluOpType
    AX = mybir.AxisListType

    TT = nc.vector.tensor_tensor
    TS = nc.vector.tensor_scalar
    STT = nc.vector.scalar_tensor_tensor
    fp32r = mybir.dt.float32r
    bf16 = mybir.dt.bfloat16

    def mmr(out, lhsT, rhs, **kw):
        # fp32r (TF32-like) would be 4x faster on PE but requires rounding
        # every producer to reduced precision; keep exact fp32.
        nc.tensor.matmul(out, lhsT, rhs, **kw)

    ctx = ExitStack()
    with ctx:
        singles = ctx.enter_context(tc.tile_pool(name="singles", bufs=1))
        work = ctx.enter_context(tc.tile_pool(name="work", bufs=3))
        fin = ctx.enter_context(tc.tile_pool(name="fin", bufs=2))
        fin1 = ctx.enter_context(tc.tile_pool(name="fin1", bufs=1))
        psum_mm = ctx.enter_context(tc.tile_pool(name="psum_mm", bufs=2, space="PSUM"))
        psum_acc = ctx.enter_context(tc.tile_pool(name="psum_acc", bufs=2, space="PSUM"))
        dscr = ctx.enter_context(tc.tile_pool(name="dscr", bufs=2, space="DRAM"))

        dma = nc.sync.dma_start

        cnt = [0]

        def ps_tile(p, f):
            cnt[0] += 1
            return psum_mm.tile([p, f], fp32, tag="ps", name=f"ps{cnt[0]}")

        # ===== iota-derived constant masks =====
        iota_i = singles.tile([128, 1], i32)
        nc.gpsimd.iota(iota_i, pattern=[[0, 1]], base=0, channel_multiplier=1)
        iota_f = singles.tile([128, 1], fp32)
        nc.vector.tensor_copy(iota_f, iota_i)

        bkf_i = singles.tile([128, 16, 8], i32)   # value b' at col (b'*8+m)
        nc.gpsimd.iota(bkf_i, pattern=[[1, 16], [0, 8]], base=0, channel_multiplier=0)
        bkf_f = singles.tile([128, 16, 8], fp32)
        nc.vector.tensor_copy(bkf_f, bkf_i)

        colf_i = singles.tile([128, 128], i32)    # value j at col j
        nc.gpsimd.iota(colf_i, pattern=[[1, 128]], base=0, channel_multiplier=0)
        colf_f = singles.tile([128, 128], fp32)
        nc.vector.tensor_copy(colf_f, colf_i)

        pmod_i = singles.tile([128, 1], i32)      # p % 16
        TS(pmod_i, iota_i, 15, None, OP.bitwise_and)
        pmod_f = singles.tile([128, 1], fp32)
        nc.vector.tensor_copy(pmod_f, pmod_i)

        ident = singles.tile([128, 128], fp32)    # identity matrix
        TS(ident, colf_f, iota_f, None, OP.is_equal)

        lhsT16 = singles.tile([16, 128], fp32)     # [b, b'*8+m] = (b'==b)
        TS(lhsT16, bkf_f.rearrange("p a b -> p (a b)")[:16], iota_f[:16], None,
           OP.is_equal)

        maskbb = singles.tile([128, 16, 8], fp32)  # [p, (b',m)] = (p%16==b')
        TS(maskbb, bkf_f, pmod_f, None, OP.is_equal)

        # per-group replication masks: repl[gg][p, (b',m)] = (p == 16gg+b')
        # bf16: the replication matmul streams x as a bf16 hi/lo pair
        # (exact to ~2^-16, vs fp32r's 2^-12 single-stream rounding)
        repl = singles.tile([128, GG, 16, 8], bf16)
        for g in range(GG):
            pg = work.tile([128, 1], fp32, tag="pg", name="pg")
            TS(pg, iota_f, float(-16 * g), None, OP.add)
            TS(repl[:, g, :, :], bkf_f, pg, None, OP.is_equal)

        ps_h16 = ps_tile(128, 16)
        nc.tensor.transpose(ps_h16, lhsT16, ident[:16, :16])
        H16 = singles.tile([128, 16], fp32)        # [p, b'] = (p//8==b')
        nc.scalar.copy(H16, ps_h16)

        # gsel[p, g] = (p//16 == g)
        pdiv16_i = singles.tile([128, 1], i32)
        TS(pdiv16_i, iota_i, 4, None, OP.arith_shift_right)
        pdiv16_f = singles.tile([128, 1], fp32)
        nc.vector.tensor_copy(pdiv16_f, pdiv16_i)
        col8_i = singles.tile([128, 8], i32)
        nc.gpsimd.iota(col8_i, pattern=[[1, 8]], base=0, channel_multiplier=0)
        col8_f = singles.tile([128, 8], fp32)
        nc.vector.tensor_copy(col8_f, col8_i)
        gsel = singles.tile([128, 8], fp32)
        TS(gsel, col8_f, pdiv16_f, None, OP.is_equal)

        # ===== weights =====
        W1s = singles.tile([CD, H], fp32)
        dma(out=W1s, in_=io["W1"][:, :])
        W2s = [singles.tile([128, H], fp32, tag=f"w2_{i}", name=f"w2_{i}") for i in range(2)]
        W3s = [singles.tile([128, OUT3], fp32, tag=f"w3_{i}", name=f"w3_{i}") for i in range(2)]
        for i in range(2):
            dma(out=W2s[i], in_=io["W2"][i * 128:(i + 1) * 128, :])
            dma(out=W3s[i], in_=io["W3"][i * 128:(i + 1) * 128, :])
        b1t = singles.tile([128, 2], fp32)
        dma(out=b1t, in_=io["b1"].rearrange("(h p) -> p h", p=128))
        b2t = singles.tile([128, 2], fp32)
        dma(out=b2t, in_=io["b2"].rearrange("(h p) -> p h", p=128))
        b3t = singles.tile([OUT3, 1], fp32)
        dma(out=b3t, in_=io["b3"].rearrange("(o u) -> o u", u=1))

        # (y is loaded per-t in the main loop; no stored xc: the fp32r stream
        # copy is clipped from y per chunk, and the finale reconstructs
        # ee = y - clip(y) exactly from y)

        # ===== MLP =====
        condT = singles.tile([CD, BL], fp32)
        for t in range(T):
            csb = work.tile([128, CD], fp32, tag="cond", name="csb")
            dma(out=csb, in_=io["cond"][t * 128:(t + 1) * 128, :])
            ps = ps_tile(CD, 128)
            nc.tensor.transpose(ps, csb, ident)
            nc.scalar.copy(condT[:, t * 128:(t + 1) * 128], ps)

        h1 = [singles.tile([128, BL], fp32, tag=f"h1_{i}", name=f"h1_{i}") for i in range(2)]
        for half in range(2):
            ps = ps_tile(128, BL)
            mmr(ps, W1s[:, half * 128:(half + 1) * 128], condT,
                start=True, stop=True)
            nc.scalar.activation(h1[half], ps, AF.Relu, bias=b1t[:, half:half + 1])
        h2 = [singles.tile([128, BL], fp32, tag=f"h2_{i}", name=f"h2_{i}") for i in range(2)]
        for half in range(2):
            ps = ps_tile(128, BL)
            for kc in range(2):
                mmr(ps, W2s[kc][:, half * 128:(half + 1) * 128], h1[kc],
                    start=(kc == 0), stop=(kc == 1))
            nc.scalar.activation(h2[half], ps, AF.Relu, bias=b2t[:, half:half + 1])
        p_f = singles.tile([OUT3, BL], fp32)
        ps49 = ps_tile(OUT3, BL)
        for kc in range(2):
            mmr(ps49, W3s[kc], h2[kc], start=(kc == 0), stop=(kc == 1))
        nc.scalar.activation(p_f, ps49, AF.Identity, bias=b3t)

        pw = singles.tile([128, T, OUT3], fp32)   # p row-major
        for t in range(T):
            ps = ps_tile(128, OUT3)
            nc.tensor.transpose(ps, p_f[:, t * 128:(t + 1) * 128], ident[:OUT3, :OUT3])
            nc.scalar.copy(pw[:, t, :], ps)

        # ===== param pipeline =====
        un_w = pw[:, :, 0:K]
        un_h = pw[:, :, K:2 * K]
        un_d = pw[:, :, 2 * K:3 * K + 1]

        def softmax_w(un, mb, tag):
            mx = singles.tile([128, T], fp32, tag=f"mx{tag}", name=f"mx{tag}")
            nc.vector.tensor_reduce(mx, un, axis=AX.X, op=OP.max)
            nmx = singles.tile([128, T], fp32, tag=f"nmx{tag}", name=f"nmx{tag}")
            TS(nmx, mx, -1.0, None, OP.mult)
            ein = singles.tile([128, T, K], fp32, tag=f"ein{tag}", name=f"ein{tag}")
            for t in range(T):
                TS(ein[:, t, :], un[:, t, :], nmx[:, t:t + 1], None, OP.add)
            ex = singles.tile([128, T, K], fp32, tag=f"ex{tag}", name=f"ex{tag}")
            nc.scalar.activation(ex, ein, AF.Exp)
            sm = singles.tile([128, T], fp32, tag=f"sm{tag}", name=f"sm{tag}")
            nc.vector.tensor_reduce(sm, ex, axis=AX.X, op=OP.add)
            rs = singles.tile([128, T], fp32, tag=f"rs{tag}", name=f"rs{tag}")
            nc.vector.reciprocal(rs, sm)
            wd = singles.tile([128, T, K], fp32, tag=f"wd{tag}", name=f"wd{tag}")
            for t in range(T):
                TS(wd[:, t, :], ex[:, t, :], rs[:, t:t + 1], 2 * BOUND - K * mb,
                   OP.mult, OP.mult)
            TS(wd, wd, mb, None, OP.add)
            return wd

        widths = softmax_w(un_w, MBW, "w")
        heights = softmax_w(un_h, MBH, "h")

        zeros16 = singles.tile([128, K], fp32)
        nc.vector.memset(zeros16, 0.0)
        cumw = singles.tile([128, T, K + 1], fp32)
        cumh = singles.tile([128, T, K + 1], fp32)
        nc.vector.memset(cumw[:, :, 0:1], -BOUND)
        nc.vector.memset(cumh[:, :, 0:1], -BOUND)
        for t in range(T):
            nc.vector.tensor_tensor_scan(cumw[:, t, 1:], widths[:, t, :], zeros16,
                                         -BOUND, OP.add, OP.add)
            nc.vector.tensor_tensor_scan(cumh[:, t, 1:], heights[:, t, :], zeros16,
                                         -BOUND, OP.add, OP.add)

        # softplus(x) = max(x,0) + ln(1 + exp(-|x|)) (no Softplus table on TRN2)
        deriv = singles.tile([128, T, K + 1], fp32)
        absd = singles.tile([128, T, K + 1], fp32)
        nc.scalar.activation(absd, un_d, AF.Abs)
        end_ = singles.tile([128, T, K + 1], fp32)
        nc.scalar.activation(end_, absd, AF.Exp, scale=-1.0)
        l1p = singles.tile([128, T, K + 1], fp32)
        nc.scalar.activation(l1p, end_, AF.Ln, bias=1.0)
        rl = singles.tile([128, T, K + 1], fp32)
        TS(rl, un_d, 0.0, MD, OP.max, OP.add)
        TT(deriv, rl, l1p, OP.add)

        d0 = deriv[:, :, 0:K]
        d1 = deriv[:, :, 1:K + 1]
        y0 = cumh[:, :, 0:K]
        kx = cumw[:, :, 0:K]
        kx1 = cumw[:, :, 1:K + 1]

        def tmp(tag):
            return singles.tile([128, T, K], fp32, tag=tag, name=tag)

        iw = tmp("iw"); nc.vector.reciprocal(iw, widths)
        delta = tmp("delta"); TT(delta, heights, iw, OP.mult)
        rdelta = tmp("rdelta"); nc.vector.reciprocal(rdelta, delta)
        # s_k = geomean(delta)/delta_k: the geomean normalization (instead of
        # delta_0) halves the dynamic range of the rescale, keeping fp32r
        # coefficient/stream rounding errors bounded
        lnd = tmp("lnd"); nc.scalar.activation(lnd, delta, AF.Ln)
        mld = singles.tile([128, T], fp32, tag="mld", name="mld")
        nc.vector.tensor_reduce(mld, lnd, axis=AX.X, op=OP.add)
        TS(mld, mld, 1.0 / K, None, OP.mult)
        gmd = singles.tile([128, T], fp32, tag="gmd", name="gmd")
        nc.scalar.activation(gmd, mld, AF.Exp)
        s = tmp("s")
        for t in range(T):
            TS(s[:, t, :], rdelta[:, t, :], gmd[:, t:t + 1], None, OP.mult)
        sig = tmp("sig"); TT(sig, d0, d1, OP.add)
        STT(sig, delta, -2.0, sig, OP.mult, OP.add)
        sdelta = tmp("sdelta"); TT(sdelta, s, delta, OP.mult)
        ssig = tmp("ssig"); TT(ssig, s, sig, OP.mult)
        sh = tmp("sh"); TT(sh, s, heights, OP.mult)
        shd0 = tmp("shd0"); TT(shd0, sh, d0, OP.mult)
        t1 = tmp("t1"); TT(t1, y0, ssig, OP.mult)
        Nc1 = tmp("Nc1"); TT(Nc1, t1, shd0, OP.add)
        u1 = tmp("u1"); TT(u1, delta, d0, OP.subtract)
        u2 = tmp("u2"); TT(u2, sh, u1, OP.mult)
        Nc2 = tmp("Nc2"); TT(Nc2, u2, t1, OP.subtract)
        sd2 = tmp("sd2"); TT(sd2, sdelta, sdelta, OP.mult)
        Cc1 = tmp("Cc1"); STT(Cc1, sd2, 2.0, u1, OP.mult, OP.mult)
        Cc2 = tmp("Cc2"); TT(Cc2, sd2, sig, OP.mult)
        iw2 = tmp("iw2"); TT(iw2, iw, iw, OP.mult)

        # final coefs into one contiguous tile: coefcat[:, t, ci, k]
        # ci: 0=aN 1=bN 2=aD 3=bD 4=aC 5=bC 6=kx 7=kx1 8=aN_lo 9=bN_lo
        # (8/9 are fp32r-rounding residuals of aN/bN: the 4th lhsT poly slot
        # accumulates them for a ~24-bit-effective N)
        coefcat = singles.tile([128, T, 10, K], fp32)
        aN = coefcat[:, :, 0, :]; TT(aN, Nc2, iw2, OP.mult)
        bN = coefcat[:, :, 1, :]; TT(bN, Nc1, iw, OP.mult)
        aD = coefcat[:, :, 2, :]; STT(aD, ssig, -1.0, iw2, OP.mult, OP.mult)
        bD = coefcat[:, :, 3, :]; TT(bD, ssig, iw, OP.mult)
        aC = coefcat[:, :, 4, :]; TT(aC, Cc2, iw2, OP.mult)
        bC = coefcat[:, :, 5, :]; TT(bC, Cc1, iw, OP.mult)
        nc.vector.tensor_copy(coefcat[:, :, 6, :], kx)
        nc.vector.tensor_copy(coefcat[:, :, 7, :], kx1)
        rndN = singles.tile([128, T, 2, K], fp32r)
        TS(rndN[:, :, 0, :], aN, 0.0, None, OP.add)
        TS(rndN[:, :, 1, :], bN, 0.0, None, OP.add)
        TT(coefcat[:, :, 8, :], aN, rndN[:, :, 0, :].bitcast(fp32), OP.subtract)
        TT(coefcat[:, :, 9, :], bN, rndN[:, :, 1, :].bitcast(fp32), OP.subtract)

        # per-row constants, packed 4-wide (pi 3 = 0) for the cpk transform
        constcat4 = singles.tile([128, 4, T], fp32)
        nc.vector.memset(constcat4[:, 3, :], 0.0)
        constN = constcat4[:, 0, :]
        TT(constN, y0[:, :, 0], sdelta[:, :, 0], OP.mult)
        constD = constcat4[:, 1, :]
        nc.vector.tensor_copy(constD, sdelta[:, :, 0])
        constC = constcat4[:, 2, :]
        TT(constC, sd2[:, :, 0], d0[:, :, 0], OP.mult)

        if MODE == "t":
            for cst, b in ((constN, bN), (constD, bD), (constC, bC)):
                bx = tmp("bx"); TT(bx, b, kx, OP.mult)
                sbx = singles.tile([128, T], fp32, tag="sbx", name="sbx")
                nc.vector.tensor_reduce(sbx, bx, axis=AX.X, op=OP.add)
                TT(cst, cst, sbx, OP.subtract)

        # ===== repack coefficients to (b*8+m) partition layout, k = 8h+m ====
        # PACKN[p=(b*8+m), t, ci, h, g] = coefcat[16g+b, t, ci, 8h+m]
        # via PE: PACK = (coef-expand * maskbb)^T @ gsel  (contraction over
        # the 128 source rows; gsel selects the group).
        NCI = 10
        PACKN = singles.tile([128, T, NCI, 2, GG], fp32)
        for t in range(T):
            psp = ps_tile(128, 128)
            pspb = ps_tile(128, 32)
            for h in range(2):
                exbig = work.tile([128, NCI, 16, 8], fp32, tag="exbig",
                                  name="exbig")
                in0 = coefcat[:, t, :, 8 * h:8 * h + 8].unsqueeze(2)\
                    .broadcast_to([128, NCI, 16, 8])
                in1 = maskbb.unsqueeze(1).broadcast_to([128, NCI, 16, 8])
                TT(exbig, in0, in1, OP.mult)
                for ci in range(NCI):
                    lhs = exbig[:, ci, :, :].rearrange("p a b -> p (a b)")
                    if ci < 8:
                        nc.tensor.matmul(
                            psp[:, (ci * 2 + h) * 8:(ci * 2 + h) * 8 + 8],
                            lhs, gsel, start=True, stop=True)
                    else:
                        nc.tensor.matmul(
                            pspb[:, ((ci - 8) * 2 + h) * 8:((ci - 8) * 2 + h) * 8 + 8],
                            lhs, gsel, start=True, stop=True)
            nc.scalar.copy(
                PACKN[:, t, 0:8, :, :].rearrange("p a b c -> p (a b c)"), psp)
            nc.scalar.copy(
                PACKN[:, t, 8:10, :, :].rearrange("p a b c -> p (a b c)"), pspb)
        NEGKX = singles.tile([128, T, 2, GG], fp32)
        TS(NEGKX, PACKN[:, :, 6, :, :], -1.0, None, OP.mult)
        PACKW = singles.tile([128, T, 2, GG], fp32)   # bin width per slot
        TT(PACKW, PACKN[:, :, 7, :, :], PACKN[:, :, 6, :, :], OP.subtract)

        # lhsT mega: [128, T, 2, GG, 4, 16]; per (t,h,g) a contiguous
        # [4poly, 16b'] = 64-col block; poly slots: 0=N 1=D 2=C 3=N_lo
        LHS_L = singles.tile([128, T, 2, GG, 4, 16], fp32r)
        LHS_Q = singles.tile([128, T, 2, GG, 4, 16], fp32r)
        for t in range(T):
            for h in range(2):
                for pi, (lin_c, sq_c) in enumerate(
                        ((1, 0), (3, 2), (5, 4), (9, 8))):
                    for dst, ci in ((LHS_L, lin_c), (LHS_Q, sq_c)):
                        csrc = PACKN[:, t, ci, h, :]  # [128, GG]
                        bcs = csrc.unsqueeze(2).broadcast_to([128, GG, 16])
                        h16b = H16.unsqueeze(1).broadcast_to([128, GG, 16])
                        TT(dst[:, t, h, :, pi, :], bcs, h16b, OP.mult)

        # ===== main loop =====
        for t in range(T):
            yt = work.tile([128, N], fp32, tag="yt", name="yt")
            dma(out=yt, in_=io["y"][t * 128:(t + 1) * 128, :])
            for c in range(CH):
                ysl = yt[:, c * 512:(c + 1) * 512]
                # exact clip (finale) + bf16 hi/lo pair for the replication
                xcf = work.tile([128, 512], fp32, tag="xcf", name="xcf")
                nc.gpsimd.tensor_scalar(xcf, ysl, -BOUND, BOUND, OP.max, OP.min)
                xch = work.tile([128, 512], bf16, tag="xch", name="xch")
                nc.scalar.copy(xch, xcf)
                xcl = work.tile([128, 512], bf16, tag="xcl", name="xcl")
                TT(xcl, xcf, xch.bitcast(bf16), OP.subtract)
                # SACC[p=(pi*16+b), g, j]: per-group drained spline polys
                SACC = fin.tile([64, GG, 512], fp32, tag="SACC", name="SACC")
                slotpair = None
                for g in range(GG):
                    xrep = psum_mm.tile([128, 512], fp32, tag="xrep", name="xrep")
                    rl_ = repl[:, g, :, :].rearrange("p a b -> p (a b)")
                    mmr(xrep, rl_, xch, start=True, stop=False)
                    mmr(xrep, rl_, xcl, start=False, stop=True)
                    # fp32r matmul dst must start at partition 0: [64,512]
                    # PSUM regions per group, paired in a [64,1024] tile so
                    # two groups drain in one op
                    if g % 2 == 0:
                        slotpair = psum_acc.tile([64, 1024], fp32, tag="slot",
                                                 name="slot")
                    slot = slotpair[:, (g % 2) * 512:(g % 2) * 512 + 512]
                    for h in range(2):
                        ulin = work.tile([128, 512], fp32r, tag="ulin",
                                         name="ulin")
                        usq = work.tile([128, 512], fp32r, tag="usq", name="usq")
                        if h == 0 and g < 7:
                            # tk-chain: DVE clip, ACT shift + ACT square
                            tk = work.tile([128, 512], fp32, tag="tk", name="tk")
                            TS(tk, xrep, PACKN[:, t, 6, h, g:g + 1],
                               PACKN[:, t, 7, h, g:g + 1], OP.max, OP.min)
                            nc.scalar.activation(ulin, tk, AF.Identity,
                                                 bias=NEGKX[:, t, h, g:g + 1])
                            nc.scalar.activation(usq, tk, AF.Square,
                                                 bias=NEGKX[:, t, h, g:g + 1])
                        else:
                            # relu-chain: ACT relu (PSUM), DVE min + DVE square
                            r_ = work.tile([128, 512], fp32, tag="tk", name="r_")
                            nc.scalar.activation(r_, xrep, AF.Relu,
                                                 bias=NEGKX[:, t, h, g:g + 1])
                            TS(ulin, r_, PACKW[:, t, h, g:g + 1], None, OP.min)
                            TT(usq, ulin.bitcast(fp32), ulin.bitcast(fp32),
                               OP.mult)
                        ll = LHS_L[:, t, h, g, :, :].rearrange("p a b -> p (a b)")
                        lq = LHS_Q[:, t, h, g, :, :].rearrange("p a b -> p (a b)")
                        mmr(slot, ll, ulin, start=(h == 0), stop=False)
                        mmr(slot, lq, usq, start=False, stop=(h == 1))
                    if g % 2 == 1:
                        # drain both groups' slots in one [64,1024] op
                        dstv = SACC[:, g - 1:g + 1, :].rearrange("p a b -> p (a b)")
                        nc.scalar.copy(dstv, slotpair)
                # bounce through DRAM to un-interleave (poly, b) rows:
                # 4 scattered writes + 4 contiguous reads (N, D, C, N_lo)
                D1 = dscr.tile([4, 128, 512], fp32, name="D1")
                for pi in range(4):
                    psrc = SACC[pi * 16:pi * 16 + 16, :, :]
                    dview = bass.AP(
                        tensor=D1.tensor,
                        offset=D1.offset + pi * 128 * 512,
                        ap=[[512, 16], [16 * 512, GG], [1, 512]])
                    dmax = dma if pi % 2 == 0 else nc.scalar.dma_start
                    dmax(out=dview, in_=psrc)
                polys = []
                for pi in range(4):
                    dstt = fin.tile([128, 512], fp32, tag=f"poly{pi}",
                                    name=f"poly{pi}")
                    dmax = dma if pi % 2 == 0 else nc.scalar.dma_start
                    dmax(out=dstt, in_=D1[pi, :, :])
                    polys.append(dstt)
                Np, Dp, Cp, Nl = polys

                # finale; per-row consts applied here (row-aligned layout)
                cN = constcat4[:, 0, t:t + 1]
                cD = constcat4[:, 1, t:t + 1]
                cC = constcat4[:, 2, t:t + 1]
                Cm = fin1.tile([128, 512], fp32, tag="Cm", name="Cm")
                nc.gpsimd.tensor_scalar(Cm, Cp, cC, 1e-12, OP.add, OP.max)
                Dc = fin1.tile([128, 512], fp32, tag="Dc", name="Dc")
                nc.gpsimd.tensor_scalar(Dc, Dp, cD, None, OP.add)
                rD = fin1.tile([128, 512], fp32, tag="rD", name="rD")
                nc.vector.reciprocal(rD, Dc)
                NpC = fin1.tile([128, 512], fp32, tag="NpC", name="NpC")
                nc.gpsimd.tensor_scalar(NpC, Np, cN, None, OP.add)
                NpF = fin1.tile([128, 512], fp32, tag="NpF", name="NpF")
                nc.gpsimd.tensor_tensor(NpF, NpC, Nl, OP.add)
                out0 = fin1.tile([128, 512], fp32, tag="out0", name="out0")
                nc.gpsimd.tensor_tensor(out0, NpF, rD, OP.mult)
                # logdet = ln(Cm * rD * rD)
                q1 = fin1.tile([128, 512], fp32, tag="q1", name="q1")
                nc.gpsimd.tensor_tensor(q1, Cm, rD, OP.mult)
                q2 = fin1.tile([128, 512], fp32, tag="q2", name="q2")
                nc.gpsimd.tensor_tensor(q2, q1, rD, OP.mult)
                ld0 = fin1.tile([128, 512], fp32, tag="ld0", name="ld0")
                nc.scalar.activation(ld0, q2, AF.Ln)
                ee = fin1.tile([128, 512], fp32, tag="ee", name="ee")
                nc.gpsimd.tensor_tensor(ee, ysl, xcf, OP.subtract)
                inz = fin1.tile([128, 512], fp32, tag="inz", name="inz")
                nc.gpsimd.tensor_scalar(inz, ee, 0.0, None, OP.is_equal)
                outF = fin1.tile([128, 512], fp32, tag="outF", name="outF")
                nc.gpsimd.tensor_tensor(outF, out0, ee, OP.add)
                ldF = fin1.tile([128, 512], fp32, tag="ldF", name="ldF")
                nc.gpsimd.tensor_tensor(ldF, ld0, inz, OP.mult)
                dma(out=io["out"][t * 128:(t + 1) * 128, c * 512:(c + 1) * 512],
                    in_=outF)
                nc.scalar.dma_start(
                    out=io["logdet"][t * 128:(t + 1) * 128, c * 512:(c + 1) * 512],
                    in_=ldF)


def kernel(cond, y, W1, b1, W2, b2, W3, b3):
    _ensure_path()
    from concourse.bass_utils import run_bass_kernel_spmd

    if "nc" not in _CACHE:
        _CACHE["nc"] = _build_nc()
    nc = _CACHE["nc"]

    cond = np.ascontiguousarray(cond, np.float32)
    y = np.ascontiguousarray(y, np.float32)
    shared = dict(W1=np.ascontiguousarray(W1, np.float32),
                  b1=np.ascontiguousarray(b1, np.float32),
                  W2=np.ascontiguousarray(W2, np.float32),
                  b2=np.ascontiguousarray(b2, np.float32),
                  W3=np.ascontiguousarray(W3, np.float32),
                  b3=np.ascontiguousarray(b3, np.float32))
    in_maps = []
    for i in range(NCORES):
        sl = slice(i * BL, (i + 1) * BL)
        in_maps.append(dict(cond=cond[sl], y=y[sl], **shared))
    res = run_bass_kernel_spmd(nc, in_maps, core_ids=list(range(NCORES)))
    out = np.concatenate([r["out"] for r in res.results], axis=0)
    ld = np.concatenate([r["logdet"] for r in res.results], axis=0)
    return out, ld



# revision 48
# speedup vs baseline: 1.0341x; 1.0341x over previous
"""Trainium2 Bass kernel for nn_ConditionalSpline1DFlow (K=16 RQS flow).

Data-parallel over 8 cores (B=4096 -> 512 rows/core). Per core:
  1. Conditioner MLP on TensorE (feature-major).
  2. Spline params per row; rescale bin k's rational-quadratic by
     s_k = delta_0/delta_k so numerator N, denominator D and
     derivative-numerator C become globally CONTINUOUS piecewise
     quadratics in x.
  3. Evaluate N, D, C gather-free in the clipped-ramp basis
        P(x) = const + sum_k a_k*(t_k - x_k)^2 + b_k*(t_k - x_k),
        t_k = clip(x, x_k, x_{k+1})
     on TensorE: rows packed (b*16+k) so one [128, 24] matmul contracts
     all 16 bins x 3 polys for 8 batch rows at once; PSUM accumulates the
     (linear, square) stream pair.
  4. out = N/D + (y - clip(y)); logdet = (ln C - 2 ln D) * (y == clip(y)).
"""
import sys
import numpy as np

K = 16
BOUND = 5.0
MBW = 1e-3
MBH = 1e-3
MD = 1e-3
B_FULL, N = 4096, 1024
CD, H = 64, 256
OUT3 = 3 * K + 1
NCORES = 8
BL = B_FULL // NCORES   # 512 rows per core
T = BL // 128           # 4 partition tiles
G = 128 // 8            # (unused) 8-row groups
GG = 128 // 16          # 8 groups of 16 rows per tile
CH = N // 512           # 2 free-dim chunks

MODE = "u"  # "t": stream clipped-t w/ folded consts; "u": stream t - x_k
            # (u-basis: streams/coefs stay O(per-bin contribution), required
            # for fp32r's ~2^-12 rounding to stay inside the error budget)

_CACHE = {}


def _ensure_path():
    for p in ("/opt/trn_rl_repo",):
        if p not in sys.path:
            sys.path.insert(0, p)


def _build_nc():
    _ensure_path()
    import concourse.bacc as bacc
    import concourse.tile as tile
    from concourse import mybir

    fp32 = mybir.dt.float32
    nc = bacc.Bacc("TRN2", target_bir_lowering=False, debug=False)

    io = dict(
        cond=nc.dram_tensor("cond", [BL, CD], fp32, kind="ExternalInput"),
        y=nc.dram_tensor("y", [BL, N], fp32, kind="ExternalInput"),
        W1=nc.dram_tensor("W1", [CD, H], fp32, kind="ExternalInput"),
        b1=nc.dram_tensor("b1", [H], fp32, kind="ExternalInput"),
        W2=nc.dram_tensor("W2", [H, H], fp32, kind="ExternalInput"),
        b2=nc.dram_tensor("b2", [H], fp32, kind="ExternalInput"),
        W3=nc.dram_tensor("W3", [H, OUT3], fp32, kind="ExternalInput"),
        b3=nc.dram_tensor("b3", [OUT3], fp32, kind="ExternalInput"),
        out=nc.dram_tensor("out", [BL, N], fp32, kind="ExternalOutput"),
        logdet=nc.dram_tensor("logdet", [BL, N], fp32, kind="ExternalOutput"),
    )
    with tile.TileContext(nc) as tc:
        _emit(nc, tc, io)
    nc.compile()
    return nc


def _emit(nc, tc, io):
    from contextlib import ExitStack
    import concourse.bass as bass
    from concourse import mybir

    fp32 = mybir.dt.float32
    i32 = mybir.dt.int32
    AF = mybir.ActivationFunctionType
    OP = mybir.AluOpType
    AX = mybir.AxisListType

    TT = nc.vector.tensor_tensor
    TS = nc.vector.tensor_scalar
    STT = nc.vector.scalar_tensor_tensor
    fp32r = mybir.dt.float32r
    bf16 = mybir.dt.bfloat16

    def mmr(out, lhsT, rhs, **kw):
        # fp32r (TF32-like) would be 4x faster on PE but requires rounding
        # every producer to reduced precision; keep exact fp32.
        nc.tensor.matmul(out, lhsT, rhs, **kw)

    ctx = ExitStack()
    with ctx:
        singles = ctx.enter_context(tc.tile_pool(name="singles", bufs=1))
        work = ctx.enter_context(tc.tile_pool(name="work", bufs=3))
        fin = ctx.enter_context(tc.tile_pool(name="fin", bufs=2))
        fin1 = ctx.enter_context(tc.tile_pool(name="fin1", bufs=1))
        psum_mm = ctx.enter_context(tc.tile_pool(name="psum_mm", bufs=2, space="PSUM"))
        psum_acc = ctx.enter_context(tc.tile_pool(name="psum_acc", bufs=2, space="PSUM"))
        dscr = ctx.enter_context(tc.tile_pool(name="dscr", bufs=2, space="DRAM"))

        dma = nc.sync.dma_start

        cnt = [0]

        def ps_tile(p, f):
            cnt[0] += 1
            return psum_mm.tile([p, f], fp32, tag="ps", name=f"ps{cnt[0]}")

        # ===== iota-derived constant masks =====
        iota_i = singles.tile([128, 1], i32)
        nc.gpsimd.iota(iota_i, pattern=[[0, 1]], base=0, channel_multiplier=1)
        iota_f = singles.tile([128, 1], fp32)
        nc.vector.tensor_copy(iota_f, iota_i)

        bkf_i = singles.tile([128, 16, 8], i32)   # value b' at col (b'*8+m)
        nc.gpsimd.iota(bkf_i, pattern=[[1, 16], [0, 8]], base=0, channel_multiplier=0)
        bkf_f = singles.tile([128, 16, 8], fp32)
        nc.vector.tensor_copy(bkf_f, bkf_i)

        colf_i = singles.tile([128, 128], i32)    # value j at col j
        nc.gpsimd.iota(colf_i, pattern=[[1, 128]], base=0, channel_multiplier=0)
        colf_f = singles.tile([128, 128], fp32)
        nc.vector.tensor_copy(colf_f, colf_i)

        pmod_i = singles.tile([128, 1], i32)      # p % 16
        TS(pmod_i, iota_i, 15, None, OP.bitwise_and)
        pmod_f = singles.tile([128, 1], fp32)
        nc.vector.tensor_copy(pmod_f, pmod_i)

        ident = singles.tile([128, 128], fp32)    # identity matrix
        TS(ident, colf_f, iota_f, None, OP.is_equal)

        lhsT16 = singles.tile([16, 128], fp32)     # [b, b'*8+m] = (b'==b)
        TS(lhsT16, bkf_f.rearrange("p a b -> p (a b)")[:16], iota_f[:16], None,
           OP.is_equal)

        maskbb = singles.tile([128, 16, 8], fp32)  # [p, (b',m)] = (p%16==b')
        TS(maskbb, bkf_f, pmod_f, None, OP.is_equal)

        # per-group replication masks: repl[gg][p, (b',m)] = (p == 16gg+b')
        # bf16: the replication matmul streams x as a bf16 hi/lo pair
        # (exact to ~2^-16, vs fp32r's 2^-12 single-stream rounding)
        repl = singles.tile([128, GG, 16, 8], bf16)
        for g in range(GG):
            pg = work.tile([128, 1], fp32, tag="pg", name="pg")
            TS(pg, iota_f, float(-16 * g), None, OP.add)
            TS(repl[:, g, :, :], bkf_f, pg, None, OP.is_equal)

        ps_h16 = ps_tile(128, 16)
        nc.tensor.transpose(ps_h16, lhsT16, ident[:16, :16])
        H16 = singles.tile([128, 16], fp32)        # [p, b'] = (p//8==b')
        nc.scalar.copy(H16, ps_h16)

        # gsel[p, g] = (p//16 == g)
        pdiv16_i = singles.tile([128, 1], i32)
        TS(pdiv16_i, iota_i, 4, None, OP.arith_shift_right)
        pdiv16_f = singles.tile([128, 1], fp32)
        nc.vector.tensor_copy(pdiv16_f, pdiv16_i)
        col8_i = singles.tile([128, 8], i32)
        nc.gpsimd.iota(col8_i, pattern=[[1, 8]], base=0, channel_multiplier=0)
        col8_f = singles.tile([128, 8], fp32)
        nc.vector.tensor_copy(col8_f, col8_i)
        gsel = singles.tile([128, 8], fp32)
        TS(gsel, col8_f, pdiv16_f, None, OP.is_equal)

        # ===== weights =====
        W1s = singles.tile([CD, H], fp32)
        dma(out=W1s, in_=io["W1"][:, :])
        W2s = [singles.tile([128, H], fp32, tag=f"w2_{i}", name=f"w2_{i}") for i in range(2)]
        W3s = [singles.tile([128, OUT3], fp32, tag=f"w3_{i}", name=f"w3_{i}") for i in range(2)]
        for i in range(2):
            dma(out=W2s[i], in_=io["W2"][i * 128:(i + 1) * 128, :])
            dma(out=W3s[i], in_=io["W3"][i * 128:(i + 1) * 128, :])
        b1t = singles.tile([128, 2], fp32)
        dma(out=b1t, in_=io["b1"].rearrange("(h p) -> p h", p=128))
        b2t = singles.tile([128, 2], fp32)
        dma(out=b2t, in_=io["b2"].rearrange("(h p) -> p h", p=128))
        b3t = singles.tile([OUT3, 1], fp32)
        dma(out=b3t, in_=io["b3"].rearrange("(o u) -> o u", u=1))

        # (y is loaded per-t in the main loop; no stored xc: the fp32r stream
        # copy is clipped from y per chunk, and the finale reconstructs
        # ee = y - clip(y) exactly from y)

        # ===== MLP =====
        condT = singles.tile([CD, BL], fp32)
        for t in range(T):
            csb = work.tile([128, CD], fp32, tag="cond", name="csb")
            dma(out=csb, in_=io["cond"][t * 128:(t + 1) * 128, :])
            ps = ps_tile(CD, 128)
            nc.tensor.transpose(ps, csb, ident)
            nc.scalar.copy(condT[:, t * 128:(t + 1) * 128], ps)

        h1 = [singles.tile([128, BL], fp32, tag=f"h1_{i}", name=f"h1_{i}") for i in range(2)]
        for half in range(2):
            ps = ps_tile(128, BL)
            mmr(ps, W1s[:, half * 128:(half + 1) * 128], condT,
                start=True, stop=True)
            nc.scalar.activation(h1[half], ps, AF.Relu, bias=b1t[:, half:half + 1])
        h2 = [singles.tile([128, BL], fp32, tag=f"h2_{i}", name=f"h2_{i}") for i in range(2)]
        for half in range(2):
            ps = ps_tile(128, BL)
            for kc in range(2):
                mmr(ps, W2s[kc][:, half * 128:(half + 1) * 128], h1[kc],
                    start=(kc == 0), stop=(kc == 1))
            nc.scalar.activation(h2[half], ps, AF.Relu, bias=b2t[:, half:half + 1])
        p_f = singles.tile([OUT3, BL], fp32)
        ps49 = ps_tile(OUT3, BL)
        for kc in range(2):
            mmr(ps49, W3s[kc], h2[kc], start=(kc == 0), stop=(kc == 1))
        nc.scalar.activation(p_f, ps49, AF.Identity, bias=b3t)

        pw = singles.tile([128, T, OUT3], fp32)   # p row-major
        for t in range(T):
            ps = ps_tile(128, OUT3)
            nc.tensor.transpose(ps, p_f[:, t * 128:(t + 1) * 128], ident[:OUT3, :OUT3])
            nc.scalar.copy(pw[:, t, :], ps)

        # ===== param pipeline =====
        un_w = pw[:, :, 0:K]
        un_h = pw[:, :, K:2 * K]
        un_d = pw[:, :, 2 * K:3 * K + 1]

        def softmax_w(un, mb, tag):
            mx = singles.tile([128, T], fp32, tag=f"mx{tag}", name=f"mx{tag}")
            nc.vector.tensor_reduce(mx, un, axis=AX.X, op=OP.max)
            nmx = singles.tile([128, T], fp32, tag=f"nmx{tag}", name=f"nmx{tag}")
            TS(nmx, mx, -1.0, None, OP.mult)
            ein = singles.tile([128, T, K], fp32, tag=f"ein{tag}", name=f"ein{tag}")
            for t in range(T):
                TS(ein[:, t, :], un[:, t, :], nmx[:, t:t + 1], None, OP.add)
            ex = singles.tile([128, T, K], fp32, tag=f"ex{tag}", name=f"ex{tag}")
            nc.scalar.activation(ex, ein, AF.Exp)
            sm = singles.tile([128, T], fp32, tag=f"sm{tag}", name=f"sm{tag}")
            nc.vector.tensor_reduce(sm, ex, axis=AX.X, op=OP.add)
            rs = singles.tile([128, T], fp32, tag=f"rs{tag}", name=f"rs{tag}")
            nc.vector.reciprocal(rs, sm)
            wd = singles.tile([128, T, K], fp32, tag=f"wd{tag}", name=f"wd{tag}")
            for t in range(T):
                TS(wd[:, t, :], ex[:, t, :], rs[:, t:t + 1], 2 * BOUND - K * mb,
                   OP.mult, OP.mult)
            TS(wd, wd, mb, None, OP.add)
            return wd

        widths = softmax_w(un_w, MBW, "w")
        heights = softmax_w(un_h, MBH, "h")

        zeros16 = singles.tile([128, K], fp32)
        nc.vector.memset(zeros16, 0.0)
        cumw = singles.tile([128, T, K + 1], fp32)
        cumh = singles.tile([128, T, K + 1], fp32)
        nc.vector.memset(cumw[:, :, 0:1], -BOUND)
        nc.vector.memset(cumh[:, :, 0:1], -BOUND)
        for t in range(T):
            nc.vector.tensor_tensor_scan(cumw[:, t, 1:], widths[:, t, :], zeros16,
                                         -BOUND, OP.add, OP.add)
            nc.vector.tensor_tensor_scan(cumh[:, t, 1:], heights[:, t, :], zeros16,
                                         -BOUND, OP.add, OP.add)

        # softplus(x) = max(x,0) + ln(1 + exp(-|x|)) (no Softplus table on TRN2)
        deriv = singles.tile([128, T, K + 1], fp32)
        absd = singles.tile([128, T, K + 1], fp32)
        nc.scalar.activation(absd, un_d, AF.Abs)
        end_ = singles.tile([128, T, K + 1], fp32)
        nc.scalar.activation(end_, absd, AF.Exp, scale=-1.0)
        l1p = singles.tile([128, T, K + 1], fp32)
        nc.scalar.activation(l1p, end_, AF.Ln, bias=1.0)
        rl = singles.tile([128, T, K + 1], fp32)
        TS(rl, un_d, 0.0, MD, OP.max, OP.add)
        TT(deriv, rl, l1p, OP.add)

        d0 = deriv[:, :, 0:K]
        d1 = deriv[:, :, 1:K + 1]
        y0 = cumh[:, :, 0:K]
        kx = cumw[:, :, 0:K]
        kx1 = cumw[:, :, 1:K + 1]

        def tmp(tag):
            return singles.tile([128, T, K], fp32, tag=tag, name=tag)

        iw = tmp("iw"); nc.vector.reciprocal(iw, widths)
        delta = tmp("delta"); TT(delta, heights, iw, OP.mult)
        rdelta = tmp("rdelta"); nc.vector.reciprocal(rdelta, delta)
        # s_k = geomean(delta)/delta_k: the geomean normalization (instead of
        # delta_0) halves the dynamic range of the rescale, keeping fp32r
        # coefficient/stream rounding errors bounded
        lnd = tmp("lnd"); nc.scalar.activation(lnd, delta, AF.Ln)
        mld = singles.tile([128, T], fp32, tag="mld", name="mld")
        nc.vector.tensor_reduce(mld, lnd, axis=AX.X, op=OP.add)
        TS(mld, mld, 1.0 / K, None, OP.mult)
        gmd = singles.tile([128, T], fp32, tag="gmd", name="gmd")
        nc.scalar.activation(gmd, mld, AF.Exp)
        s = tmp("s")
        for t in range(T):
            TS(s[:, t, :], rdelta[:, t, :], gmd[:, t:t + 1], None, OP.mult)
        sig = tmp("sig"); TT(sig, d0, d1, OP.add)
        STT(sig, delta, -2.0, sig, OP.mult, OP.add)
        sdelta = tmp("sdelta"); TT(sdelta, s, delta, OP.mult)
        ssig = tmp("ssig"); TT(ssig, s, sig, OP.mult)
        sh = tmp("sh"); TT(sh, s, heights, OP.mult)
        shd0 = tmp("shd0"); TT(shd0, sh, d0, OP.mult)
        t1 = tmp("t1"); TT(t1, y0, ssig, OP.mult)
        Nc1 = tmp("Nc1"); TT(Nc1, t1, shd0, OP.add)
        u1 = tmp("u1"); TT(u1, delta, d0, OP.subtract)
        u2 = tmp("u2"); TT(u2, sh, u1, OP.mult)
        Nc2 = tmp("Nc2"); TT(Nc2, u2, t1, OP.subtract)
        sd2 = tmp("sd2"); TT(sd2, sdelta, sdelta, OP.mult)
        Cc1 = tmp("Cc1"); STT(Cc1, sd2, 2.0, u1, OP.mult, OP.mult)
        Cc2 = tmp("Cc2"); TT(Cc2, sd2, sig, OP.mult)
        iw2 = tmp("iw2"); TT(iw2, iw, iw, OP.mult)

        # final coefs into one contiguous tile: coefcat[:, t, ci, k]
        # ci: 0=aN 1=bN 2=aD 3=bD 4=aC 5=bC 6=kx 7=kx1 8=aN_lo 9=bN_lo
        # (8/9 are fp32r-rounding residuals of aN/bN: the 4th lhsT poly slot
        # accumulates them for a ~24-bit-effective N)
        coefcat = singles.tile([128, T, 10, K], fp32)
        aN = coefcat[:, :, 0, :]; TT(aN, Nc2, iw2, OP.mult)
        bN = coefcat[:, :, 1, :]; TT(bN, Nc1, iw, OP.mult)
        aD = coefcat[:, :, 2, :]; STT(aD, ssig, -1.0, iw2, OP.mult, OP.mult)
        bD = coefcat[:, :, 3, :]; TT(bD, ssig, iw, OP.mult)
        aC = coefcat[:, :, 4, :]; TT(aC, Cc2, iw2, OP.mult)
        bC = coefcat[:, :, 5, :]; TT(bC, Cc1, iw, OP.mult)
        nc.vector.tensor_copy(coefcat[:, :, 6, :], kx)
        nc.vector.tensor_copy(coefcat[:, :, 7, :], kx1)
        rndN = singles.tile([128, T, 2, K], fp32r)
        TS(rndN[:, :, 0, :], aN, 0.0, None, OP.add)
        TS(rndN[:, :, 1, :], bN, 0.0, None, OP.add)
        TT(coefcat[:, :, 8, :], aN, rndN[:, :, 0, :].bitcast(fp32), OP.subtract)
        TT(coefcat[:, :, 9, :], bN, rndN[:, :, 1, :].bitcast(fp32), OP.subtract)

        # per-row constants, packed 4-wide (pi 3 = 0) for the cpk transform
        constcat4 = singles.tile([128, 4, T], fp32)
        nc.vector.memset(constcat4[:, 3, :], 0.0)
        constN = constcat4[:, 0, :]
        TT(constN, y0[:, :, 0], sdelta[:, :, 0], OP.mult)
        constD = constcat4[:, 1, :]
        nc.vector.tensor_copy(constD, sdelta[:, :, 0])
        constC = constcat4[:, 2, :]
        TT(constC, sd2[:, :, 0], d0[:, :, 0], OP.mult)

        if MODE == "t":
            for cst, b in ((constN, bN), (constD, bD), (constC, bC)):
                bx = tmp("bx"); TT(bx, b, kx, OP.mult)
                sbx = singles.tile([128, T], fp32, tag="sbx", name="sbx")
                nc.vector.tensor_reduce(sbx, bx, axis=AX.X, op=OP.add)
                TT(cst, cst, sbx, OP.subtract)

        # ===== repack coefficients to (b*8+m) partition layout, k = 8h+m ====
        # PACKN[p=(b*8+m), t, ci, h, g] = coefcat[16g+b, t, ci, 8h+m]
        # via PE: PACK = (coef-expand * maskbb)^T @ gsel  (contraction over
        # the 128 source rows; gsel selects the group).
        NCI = 10
        PACKN = singles.tile([128, T, NCI, 2, GG], fp32)
        for t in range(T):
            psp = ps_tile(128, 128)
            pspb = ps_tile(128, 32)
            for h in range(2):
                exbig = work.tile([128, NCI, 16, 8], fp32, tag="exbig",
                                  name="exbig")
                in0 = coefcat[:, t, :, 8 * h:8 * h + 8].unsqueeze(2)\
                    .broadcast_to([128, NCI, 16, 8])
                in1 = maskbb.unsqueeze(1).broadcast_to([128, NCI, 16, 8])
                tteng = TT if t < 2 else nc.gpsimd.tensor_tensor
                tteng(exbig, in0, in1, OP.mult)
                for ci in range(NCI):
                    lhs = exbig[:, ci, :, :].rearrange("p a b -> p (a b)")
                    if ci < 8:
                        nc.tensor.matmul(
                            psp[:, (ci * 2 + h) * 8:(ci * 2 + h) * 8 + 8],
                            lhs, gsel, start=True, stop=True)
                    else:
                        nc.tensor.matmul(
                            pspb[:, ((ci - 8) * 2 + h) * 8:((ci - 8) * 2 + h) * 8 + 8],
                            lhs, gsel, start=True, stop=True)
            nc.scalar.copy(
                PACKN[:, t, 0:8, :, :].rearrange("p a b c -> p (a b c)"), psp)
            nc.scalar.copy(
                PACKN[:, t, 8:10, :, :].rearrange("p a b c -> p (a b c)"), pspb)
        NEGKX = singles.tile([128, T, 2, GG], fp32)
        TS(NEGKX, PACKN[:, :, 6, :, :], -1.0, None, OP.mult)
        PACKW = singles.tile([128, T, 2, GG], fp32)   # bin width per slot
        TT(PACKW, PACKN[:, :, 7, :, :], PACKN[:, :, 6, :, :], OP.subtract)

        # lhsT mega: [128, T, 2, GG, 4, 16]; per (t,h,g) a contiguous
        # [4poly, 16b'] = 64-col block; poly slots: 0=N 1=D 2=C 3=N_lo
        LHS_L = singles.tile([128, T, 2, GG, 4, 16], fp32r)
        LHS_Q = singles.tile([128, T, 2, GG, 4, 16], fp32r)
        for t in range(T):
            for h in range(2):
                for pi, (lin_c, sq_c) in enumerate(
                        ((1, 0), (3, 2), (5, 4), (9, 8))):
                    for dst, ci in ((LHS_L, lin_c), (LHS_Q, sq_c)):
                        csrc = PACKN[:, t, ci, h, :]  # [128, GG]
                        bcs = csrc.unsqueeze(2).broadcast_to([128, GG, 16])
                        h16b = H16.unsqueeze(1).broadcast_to([128, GG, 16])
                        TT(dst[:, t, h, :, pi, :], bcs, h16b, OP.mult)

        # ===== main loop =====
        for t in range(T):
            yt = work.tile([128, N], fp32, tag="yt", name="yt")
            dma(out=yt, in_=io["y"][t * 128:(t + 1) * 128, :])
            for c in range(CH):
                ysl = yt[:, c * 512:(c + 1) * 512]
                # exact clip (finale) + bf16 hi/lo pair for the replication
                xcf = work.tile([128, 512], fp32, tag="xcf", name="xcf")
                nc.gpsimd.tensor_scalar(xcf, ysl, -BOUND, BOUND, OP.max, OP.min)
                xch = work.tile([128, 512], bf16, tag="xch", name="xch")
                nc.scalar.copy(xch, xcf)
                xcl = work.tile([128, 512], bf16, tag="xcl", name="xcl")
                TT(xcl, xcf, xch.bitcast(bf16), OP.subtract)
                # SACC[p=(pi*16+b), g, j]: per-group drained spline polys
                SACC = fin.tile([64, GG, 512], fp32, tag="SACC", name="SACC")
                slotpair = None
                pend = []
                for g in range(GG):
                    xrep = psum_mm.tile([128, 512], fp32, tag="xrep", name="xrep")
                    rl_ = repl[:, g, :, :].rearrange("p a b -> p (a b)")
                    mmr(xrep, rl_, xch, start=True, stop=False)
                    mmr(xrep, rl_, xcl, start=False, stop=True)
                    # fp32r matmul dst must start at partition 0: [64,512]
                    # PSUM regions per group, paired in a [64,1024] tile so
                    # two groups drain in one op
                    if g % 2 == 0:
                        slotpair = psum_acc.tile([64, 1024], fp32, tag="slot",
                                                 name="slot")
                    slot = slotpair[:, (g % 2) * 512:(g % 2) * 512 + 512]
                    for h in range(2):
                        ulin = work.tile([128, 512], fp32r, tag="ulin",
                                         name="ulin")
                        usq = work.tile([128, 512], fp32r, tag="usq", name="usq")
                        if h == 0 and g < 7:
                            # tk-chain: DVE clip, ACT shift + ACT square
                            tk = work.tile([128, 512], fp32, tag="tk", name="tk")
                            TS(tk, xrep, PACKN[:, t, 6, h, g:g + 1],
                               PACKN[:, t, 7, h, g:g + 1], OP.max, OP.min)
                            nc.scalar.activation(ulin, tk, AF.Identity,
                                                 bias=NEGKX[:, t, h, g:g + 1])
                            nc.scalar.activation(usq, tk, AF.Square,
                                                 bias=NEGKX[:, t, h, g:g + 1])
                        else:
                            # relu-chain: ACT relu (PSUM), DVE min + DVE square
                            r_ = work.tile([128, 512], fp32, tag="tk", name="r_")
                            nc.scalar.activation(r_, xrep, AF.Relu,
                                                 bias=NEGKX[:, t, h, g:g + 1])
                            TS(ulin, r_, PACKW[:, t, h, g:g + 1], None, OP.min)
                            TT(usq, ulin.bitcast(fp32), ulin.bitcast(fp32),
                               OP.mult)
                        ll = LHS_L[:, t, h, g, :, :].rearrange("p a b -> p (a b)")
                        lq = LHS_Q[:, t, h, g, :, :].rearrange("p a b -> p (a b)")
                        mmr(slot, ll, ulin, start=(h == 0), stop=False)
                        mmr(slot, lq, usq, start=False, stop=(h == 1))
                    if g % 2 == 1:
                        # drain both groups' slots in one [64,1024] op;
                        # emission delayed one pair so the engine-queue wait
                        # overlaps the next pair's compute
                        pend.append((SACC[:, g - 1:g + 1, :]
                                     .rearrange("p a b -> p (a b)"), slotpair))
                        if len(pend) == 2:
                            dstv0, sp0 = pend.pop(0)
                            nc.scalar.copy(dstv0, sp0)
                for dstv0, sp0 in pend:
                    nc.scalar.copy(dstv0, sp0)
                # bounce through DRAM to un-interleave (poly, b) rows:
                # 4 scattered writes + 4 contiguous reads (N, D, C, N_lo)
                D1 = dscr.tile([4, 128, 512], fp32, name="D1")
                for pi in range(4):
                    psrc = SACC[pi * 16:pi * 16 + 16, :, :]
                    dview = bass.AP(
                        tensor=D1.tensor,
                        offset=D1.offset + pi * 128 * 512,
                        ap=[[512, 16], [16 * 512, GG], [1, 512]])
                    dmax = dma if pi % 2 == 0 else nc.scalar.dma_start
                    dmax(out=dview, in_=psrc)
                polys = []
                for pi in range(4):
                    dstt = fin.tile([128, 512], fp32, tag=f"poly{pi}",
                                    name=f"poly{pi}")
                    dmax = dma if pi % 2 == 0 else nc.scalar.dma_start
                    dmax(out=dstt, in_=D1[pi, :, :])
                    polys.append(dstt)
                Np, Dp, Cp, Nl = polys

                # finale; per-row consts applied here (row-aligned layout).
                # Two short parallel chains; late tiles reuse dead poly
                # buffers (pool rotation = free double buffering).
                cN = constcat4[:, 0, t:t + 1]
                cD = constcat4[:, 1, t:t + 1]
                cC = constcat4[:, 2, t:t + 1]
                Cm = fin1.tile([128, 512], fp32, tag="Cm", name="Cm")
                nc.gpsimd.tensor_scalar(Cm, Cp, cC, 1e-12, OP.add, OP.max)
                Dc = fin1.tile([128, 512], fp32, tag="Dc", name="Dc")
                nc.gpsimd.tensor_scalar(Dc, Dp, cD, None, OP.add)
                rD = fin1.tile([128, 512], fp32, tag="rD", name="rD")
                nc.vector.reciprocal(rD, Dc)
                logD = fin1.tile([128, 512], fp32, tag="logD", name="logD")
                nc.scalar.activation(logD, Dc, AF.Ln)
                logC = fin1.tile([128, 512], fp32, tag="logC", name="logC")
                nc.scalar.activation(logC, Cm, AF.Ln)
                NlC = fin1.tile([128, 512], fp32, tag="NlC", name="NlC")
                nc.gpsimd.tensor_scalar(NlC, Nl, cN, None, OP.add)
                NpF = fin1.tile([128, 512], fp32, tag="NpF", name="NpF")
                nc.gpsimd.tensor_tensor(NpF, Np, NlC, OP.add)
                ee = fin1.tile([128, 512], fp32, tag="ee", name="ee")
                nc.gpsimd.tensor_tensor(ee, ysl, xcf, OP.subtract)
                inz = fin1.tile([128, 512], fp32, tag="inz", name="inz")
                nc.gpsimd.tensor_scalar(inz, ee, 0.0, None, OP.is_equal)
                out0 = fin.tile([128, 512], fp32, tag="poly2", name="out0")
                nc.gpsimd.tensor_tensor(out0, NpF, rD, OP.mult)
                ld0 = fin.tile([128, 512], fp32, tag="poly3", name="ld0")
                STT(ld0, logD, -2.0, logC, OP.mult, OP.add)
                outF = fin.tile([128, 512], fp32, tag="poly0", name="outF")
                nc.gpsimd.tensor_tensor(outF, out0, ee, OP.add)
                ldF = fin.tile([128, 512], fp32, tag="poly1", name="ldF")
                nc.gpsimd.tensor_tensor(ldF, ld0, inz, OP.mult)
                dma(out=io["out"][t * 128:(t + 1) * 128, c * 512:(c + 1) * 512],
                    in_=outF)
                nc.scalar.dma_start(
                    out=io["logdet"][t * 128:(t + 1) * 128, c * 512:(c + 1) * 512],
                    in_=ldF)


def kernel(cond, y, W1, b1, W2, b2, W3, b3):
    _ensure_path()
    from concourse.bass_utils import run_bass_kernel_spmd

    if "nc" not in _CACHE:
        _CACHE["nc"] = _build_nc()
    nc = _CACHE["nc"]

    cond = np.ascontiguousarray(cond, np.float32)
    y = np.ascontiguousarray(y, np.float32)
    shared = dict(W1=np.ascontiguousarray(W1, np.float32),
                  b1=np.ascontiguousarray(b1, np.float32),
                  W2=np.ascontiguousarray(W2, np.float32),
                  b2=np.ascontiguousarray(b2, np.float32),
                  W3=np.ascontiguousarray(W3, np.float32),
                  b3=np.ascontiguousarray(b3, np.float32))
    in_maps = []
    for i in range(NCORES):
        sl = slice(i * BL, (i + 1) * BL)
        in_maps.append(dict(cond=cond[sl], y=y[sl], **shared))
    res = run_bass_kernel_spmd(nc, in_maps, core_ids=list(range(NCORES)))
    out = np.concatenate([r["out"] for r in res.results], axis=0)
    ld = np.concatenate([r["logdet"] for r in res.results], axis=0)
    return out, ld

